# revision 5
# baseline (speedup 1.0000x reference)
# GRU decoder kernel for Trainium2 (Bass/Tile), data-parallel over batch.
#
# Problem (per reference):
#   h0 = tanh(latent @ Wd + bd)                      [B, H]
#   x  = latent @ W + b[0]; xz, xr, xh = split(x, 3) [B, 3H]
#   for t in range(T):   (reset_after GRU, recurrent bias b[1])
#       rec = h @ U + b[1]; rz, rr, rh = split(rec, 3)
#       z = sigmoid(xz + rz); r = sigmoid(xr + rr)
#       hh = tanh(xh + r * rh)
#       h = z*h + (1-z)*hh        -> out[:, t, :]
#
# Sharding: batch 1024 -> 8 cores x 128 rows. Weights replicated. The T loop
# runs locally per core; no collectives.
#
# Layout: FEATURE-major ("transposed") recurrence in fp16.  h lives as
# [128 feat-in-chunk (partitions), (4 chunks x batch) (free)] f16, so the
# per-step h @ U matmuls take U-chunks as the stationary operand and h as the
# MOVING operand -- matmul cost scales with the batch (free) size, and the
# recurrence needs no PE transposes / PSUM copies on the critical path.  The
# batch-major f32 output is produced off the critical path via one DMA-engine
# transpose (f16) + GPSIMD f16->f32 convert + DMA.
#
# The per-core batch of 128 is split into two halves of 64 that run the same
# recurrence software-pipelined: half A's ACT/DVE tail overlaps half B's PE
# burst, keeping PE (the bottleneck at ~3.2us/step) nearly 100% busy.
#
# Per half-step (s in {0,1}), program order per engine:
#   PE : for g in (h, r, z): inject-matmul (x_g / b1h, start=True over the
#        whole [128,256] gate bank), then 16 accumulating matmuls
#        (4 feat-chunks j x 4 K-chunks k): ps_g[:, 64j:64j+64] +=
#        U[k, g-chunk-j].T @ h[k-chunk][:, half]  (f16, N=64)
#   ACT: r = sigmoid(ps_r), z = sigmoid(ps_z), hh = tanh(t2)   [f16 out]
#   DVE: t1 = r*ps_h, t2 = t1+xh, zc = 1-z, c1 = z*h, d = zc*hh,
#        hn = c1+d, copy hn -> combined out tile                [f16, 2x/4x]
#   DMA: one dma transpose (feat-major -> batch-major f16), GP converts to
#        f32, DMA to out[:, t, :].
#
# PSUM budget (8 banks, bank-granular allocation): 6 gate tiles [128,256]
# (one bank each); the prologue x/h0 projections run feature-chunked through
# the same 6 tiles before the loop starts.

import numpy as np

B, LD, H, T_DEF = 1024, 256, 512, 128
H3 = 3 * H
NCORES = 8
BS = B // NCORES  # 128 batch rows per core
HB = BS // 2      # 64-row half-batch

_BUILD_CACHE = {}


def _build(T):
    import concourse.bass as bass
    import concourse.mybir as mybir
    import concourse.tile as tile
    from concourse import bacc
    from concourse.masks import make_identity

    f32 = mybir.dt.float32
    f16 = mybir.dt.float16
    AF = mybir.ActivationFunctionType
    OP = mybir.AluOpType

    nc = bacc.Bacc(None, target_bir_lowering=False, debug=False)

    # ---- dram inputs (host pre-formats dtypes/layouts; see kernel()) ------
    latT = nc.dram_tensor("latT", [2, 128, BS], f16, kind="ExternalInput")
    wd_d = nc.dram_tensor("wd", [2, 128, H], f16, kind="ExternalInput")
    w_d = nc.dram_tensor("w", [2, 128, H3], f16, kind="ExternalInput")
    u_d = nc.dram_tensor("u", [4, 128, H3], f16, kind="ExternalInput")
    bx_d = nc.dram_tensor("bx", [H3], f16, kind="ExternalInput")
    bd_d = nc.dram_tensor("bd", [H], f16, kind="ExternalInput")
    b1h_d = nc.dram_tensor("b1h", [128, 4 * HB], f16, kind="ExternalInput")
    out_d = nc.dram_tensor("out", [BS, T, H], f32, kind="ExternalOutput")

    # gate column ranges in the 3H axis (reference order: z, r, h)
    GSL = (slice(0, H), slice(H, 2 * H), slice(2 * H, H3))  # z, r, h

    def as3(tile_or_ap):
        return tile_or_ap[:].rearrange("p (c b) -> p c b", c=4)

    with tile.TileContext(nc) as tc:
        with (
            tc.tile_pool(name="singles", bufs=1) as singles,
            tc.tile_pool(name="work", bufs=3) as work,
            tc.tile_pool(name="hpool", bufs=3) as hpool,
            tc.tile_pool(name="opool", bufs=8) as opool,
            tc.tile_pool(name="psg", bufs=1, space="PSUM") as psg,
        ):
            # ---- load constants ------------------------------------------
            lat = [singles.tile([128, BS], f16, tag=f"lat{j}", name=f"lat{j}")
                   for j in range(2)]
            wd = [singles.tile([128, H], f16, tag=f"wd{j}", name=f"wd{j}")
                  for j in range(2)]
            w = [singles.tile([128, H3], f16, tag=f"w{j}", name=f"w{j}")
                 for j in range(2)]
            for j in range(2):
                nc.sync.dma_start(out=lat[j], in_=latT[j, :, :])
                nc.sync.dma_start(out=wd[j], in_=wd_d[j, :, :])
                nc.sync.dma_start(out=w[j], in_=w_d[j, :, :])
            u = [singles.tile([128, H3], f16, tag=f"u{k}", name=f"u{k}")
                 for k in range(4)]
            for k in range(4):
                nc.sync.dma_start(out=u[k], in_=u_d[k, :, :])

            def bcast(handle, n):
                ap = handle[:]
                return bass.AP(tensor=ap.tensor, offset=ap.offset,
                               ap=[[0, 128], [1, n]])

            xbias = singles.tile([128, H3], f16, tag="xbias")
            nc.gpsimd.dma_start(out=xbias, in_=bcast(bx_d, H3))
            bdt = singles.tile([128, H], f16, tag="bdt")
            nc.gpsimd.dma_start(out=bdt, in_=bcast(bd_d, H))
            # b1h in feature-major half layout: b1h2[p, 64j+b] = b1[2H+128j+p]
            b1h2 = singles.tile([128, 4 * HB], f16, tag="b1h2")
            nc.gpsimd.dma_start(out=b1h2, in_=b1h_d[:, :])

            ident = singles.tile([128, 128], f32, tag="ident")
            make_identity(nc, ident)
            id16 = singles.tile([128, 128], f16, tag="id16")
            nc.scalar.copy(id16, ident)

            # loop/prologue PSUM: one [128, 4*HB] f32 bank per gate per half
            def gate_ps(s):
                return [psg.tile([128, 4 * HB], f32, tag=f"ps{g}_{s}",
                                 name=f"ps{g}_{s}") for g in range(3)]

            # ---- prologue: x-projection and h0, feature-chunked ----------
            # x[g] = latent @ W[:, gate g] + bx[gate g]   (batch-major, f16)
            xg16 = [singles.tile([128, H], f16, tag=f"xg{g}", name=f"xg{g}")
                    for g in range(3)]
            for g in range(3):
                for fc in range(2):
                    ps = psg.tile([128, 4 * HB], f32, tag=f"ps{g}_{fc}",
                                  name=f"pre_x{g}_{fc}")
                    cs = slice(GSL[g].start + 256 * fc,
                               GSL[g].start + 256 * (fc + 1))
                    nc.tensor.matmul(ps, id16, xbias[:, cs], start=True, stop=False)
                    nc.tensor.matmul(ps, lat[0], w[0][:, cs], start=False, stop=False)
                    nc.tensor.matmul(ps, lat[1], w[1][:, cs], start=False, stop=True)
                    nc.scalar.copy(xg16[g][:, 256 * fc : 256 * (fc + 1)], ps)
            # h0 = tanh(latent @ Wd + bd)   (batch-major, f16)
            h016 = singles.tile([128, H], f16, tag="h016")
            for fc in range(2):
                ps = psg.tile([128, 4 * HB], f32, tag=f"ps0_{fc}",
                              name=f"pre_h0_{fc}")
                cs = slice(256 * fc, 256 * (fc + 1))
                nc.tensor.matmul(ps, id16, bdt[:, cs], start=True, stop=False)
                nc.tensor.matmul(ps, lat[0], wd[0][:, cs], start=False, stop=False)
                nc.tensor.matmul(ps, lat[1], wd[1][:, cs], start=False, stop=True)
                nc.scalar.activation(h016[:, cs], ps, AF.Tanh)

            # transpose x and h0 into feature-major half tiles:
            #   xT[g][s][p, 64j+b] = x[g][batch 64s+b, feat 128j+p]
            xT = [[singles.tile([128, 4 * HB], f16, tag=f"xT{g}_{s}",
                                name=f"xT{g}_{s}") for s in range(2)]
                  for g in range(3)]
            for g in range(3):
                for s in range(2):
                    nc.sync.dma_start_transpose(
                        as3(xT[g][s]), xg16[g][HB * s : HB * (s + 1), :])
            hc = []
            for s in range(2):
                t0 = hpool.tile([128, 4 * HB], f16, tag=f"hc{s}", name=f"h0c{s}")
                nc.sync.dma_start_transpose(
                    as3(t0), h016[HB * s : HB * (s + 1), :])
                hc.append(t0)

            # U column slice for gate g, feature-chunk j (stationary operand)
            def u_cols(gsl, j):
                base = gsl.start + 128 * j
                return slice(base, base + 128)

            # ---- steady-state T loop -------------------------------------
            for t in range(T):
                hn = [None, None]
                hcO = opool.tile([128, 4 * BS], f16, tag="hcO")
                for s in range(2):
                    hsrc = hc[s]
                    ps_z, ps_r, ps_h = gate_ps(s)

                    # PE bursts: h first (t1 needs it early), then r, then z
                    for ps, gsl, xin in ((ps_h, GSL[2], b1h2),
                                         (ps_r, GSL[1], xT[1][s]),
                                         (ps_z, GSL[0], xT[0][s])):
                        nc.tensor.matmul(ps, id16, xin, start=True, stop=False)
                        for j in range(4):
                            cs = slice(HB * j, HB * (j + 1))
                            for k in range(4):
                                nc.tensor.matmul(
                                    ps[:, cs], u[k][:, u_cols(gsl, j)],
                                    hsrc[:, HB * k : HB * (k + 1)],
                                    start=False, stop=(k == 3))

                    # ACT: r, z sigmoids then hh tanh (f16 outputs)
                    r16 = work.tile([128, 4 * HB], f16, tag=f"r16_{s}")
                    nc.scalar.activation(r16, ps_r, AF.Sigmoid)
                    z16 = work.tile([128, 4 * HB], f16, tag=f"z16_{s}")
                    nc.scalar.activation(z16, ps_z, AF.Sigmoid)

                    # DVE tail
                    t1 = work.tile([128, 4 * HB], f16, tag=f"t1_{s}")
                    nc.vector.tensor_mul(t1, r16, ps_h)
                    t2 = work.tile([128, 4 * HB], f16, tag=f"t2_{s}")
                    nc.vector.tensor_add(t2, t1, xT[2][s])
                    hh = work.tile([128, 4 * HB], f16, tag=f"hh_{s}")
                    nc.scalar.activation(hh, t2, AF.Tanh)
                    zc = work.tile([128, 4 * HB], f16, tag=f"zc_{s}")
                    nc.vector.tensor_scalar(zc, z16, -1.0, 1.0, OP.mult, OP.add)
                    c1 = work.tile([128, 4 * HB], f16, tag=f"c1_{s}")
                    nc.vector.tensor_mul(c1, z16, hsrc)
                    d = work.tile([128, 4 * HB], f16, tag=f"d_{s}")
                    nc.vector.tensor_mul(d, zc, hh)
                    hnew = hpool.tile([128, 4 * HB], f16, tag=f"hc{s}",
                                      name=f"hn{s}_{t}")
                    nc.vector.tensor_add(hnew, c1, d)
                    # merge into the combined tile for the output transpose
                    nc.vector.tensor_copy(
                        as3(hcO)[:, :, HB * s : HB * (s + 1)], as3(hnew))
                    hn[s] = hnew

                # output path (off the recurrence): f16 transpose to
                # batch-major, GP converts to f32, DMA out
                houtT = opool.tile([128, 4, 128], f16, tag="houtT")
                nc.sync.dma_start_transpose(houtT, hcO)
                hout32 = opool.tile([128, H], f32, tag="hout32")
                nc.gpsimd.tensor_copy(
                    hout32, houtT[:].rearrange("p c b -> p (c b)"))
                nc.gpsimd.dma_start(out=out_d[:, t, :], in_=hout32)

                hc = hn

    nc.compile()
    return nc


def kernel(latent, Wd, bd, W, U, b, T, _trace=False):
    from concourse.bass_utils import run_bass_kernel_spmd

    latent = np.ascontiguousarray(np.asarray(latent, dtype=np.float32))
    Wd = np.asarray(Wd, dtype=np.float32)
    bd = np.asarray(bd, dtype=np.float32)
    W = np.asarray(W, dtype=np.float32)
    U = np.asarray(U, dtype=np.float32)
    b = np.asarray(b, dtype=np.float32)
    T = int(T)

    key = (T,)
    if key not in _BUILD_CACHE:
        _BUILD_CACHE[key] = _build(T)
    nc = _BUILD_CACHE[key]

    # b[1]'s z/r thirds fold into the x-side bias; the h third stays separate
    bx = b[0].copy()
    bx[: 2 * H] += b[1][: 2 * H]
    b1h = b[1][2 * H :]
    # b1h in feature-major half layout: [p, 64j+b] = b1h[128j+p]
    b1h2 = np.ascontiguousarray(
        np.broadcast_to(b1h.reshape(4, 128).T[:, :, None],
                        (128, 4, HB)).reshape(128, 4 * HB).astype(np.float16))

    u16 = np.ascontiguousarray(U.reshape(4, 128, H3).astype(np.float16))
    wd16 = np.ascontiguousarray(Wd.reshape(2, 128, H).astype(np.float16))
    w16 = np.ascontiguousarray(W.reshape(2, 128, H3).astype(np.float16))
    bx16 = bx.astype(np.float16)
    bd16 = bd.astype(np.float16)

    in_maps = []
    for c in range(NCORES):
        rows = slice(c * BS, (c + 1) * BS)
        latTc = np.ascontiguousarray(
            latent[rows].T.reshape(2, 128, BS).astype(np.float16))
        in_maps.append({
            "latT": latTc, "wd": wd16, "w": w16, "u": u16,
            "bx": bx16, "bd": bd16, "b1h": b1h2,
        })

    res = run_bass_kernel_spmd(nc, in_maps, core_ids=list(range(NCORES)),
                               trace=_trace)
    if _trace and res.exec_time_ns is not None:
        print(f"HW exec time: {res.exec_time_ns} ns")
        if res.instructions_and_trace is not None:
            print(f"trace: {res.instructions_and_trace[1]}")

    out = np.concatenate([r["out"] for r in res.results], axis=0)
    return out


# revision 7
# speedup vs baseline: 1.0736x; 1.0736x over previous
# GRU decoder kernel for Trainium2 (Bass/Tile), data-parallel over batch.
#
# Problem (per reference):
#   h0 = tanh(latent @ Wd + bd)                      [B, H]
#   x  = latent @ W + b[0]; xz, xr, xh = split(x, 3) [B, 3H]
#   for t in range(T):   (reset_after GRU, recurrent bias b[1])
#       rec = h @ U + b[1]; rz, rr, rh = split(rec, 3)
#       z = sigmoid(xz + rz); r = sigmoid(xr + rr)
#       hh = tanh(xh + r * rh)
#       h = z*h + (1-z)*hh        -> out[:, t, :]
#
# Sharding: batch 1024 -> 8 cores x 128 rows. Weights replicated. The T loop
# runs locally per core; no collectives.
#
# Layout: FEATURE-major ("transposed") recurrence in fp16.  h lives as
# [128 feat-in-chunk (partitions), (4 chunks x batch) (free)] f16, so the
# per-step h @ U matmuls take U-chunks as the stationary operand and h as the
# MOVING operand -- matmul cost scales with the batch (free) size, and the
# recurrence needs no PE transposes / PSUM copies on the critical path.  The
# batch-major f32 output is produced off the critical path via one DMA-engine
# transpose (f16) + GPSIMD f16->f32 convert + DMA.
#
# The per-core batch of 128 is split into two halves of 64 that run the same
# recurrence software-pipelined: half A's ACT/DVE tail overlaps half B's PE
# burst, keeping PE (the bottleneck at ~3.2us/step) nearly 100% busy.
#
# Per half-step (s in {0,1}), program order per engine:
#   PE : for g in (h, r, z): inject-matmul (x_g / b1h, start=True over the
#        whole [128,256] gate bank), then 16 accumulating matmuls
#        (4 feat-chunks j x 4 K-chunks k): ps_g[:, 64j:64j+64] +=
#        U[k, g-chunk-j].T @ h[k-chunk][:, half]  (f16, N=64)
#   ACT: r = sigmoid(ps_r), z = sigmoid(ps_z), hh = tanh(t2)   [f16 out]
#   DVE: t1 = r*ps_h, t2 = t1+xh, zc = 1-z, c1 = z*h, d = zc*hh,
#        hn = c1+d, copy hn -> combined out tile                [f16, 2x/4x]
#   DMA: one dma transpose (feat-major -> batch-major f16), GP converts to
#        f32, DMA to out[:, t, :].
#
# PSUM budget (8 banks, bank-granular allocation): 6 gate tiles [128,256]
# (one bank each); the prologue x/h0 projections run feature-chunked through
# the same 6 tiles before the loop starts.

import numpy as np

B, LD, H, T_DEF = 1024, 256, 512, 128
H3 = 3 * H
NCORES = 8
BS = B // NCORES  # 128 batch rows per core
HB = BS // 2      # 64-row half-batch

_BUILD_CACHE = {}


def _build(T):
    import concourse.bass as bass
    import concourse.mybir as mybir
    import concourse.tile as tile
    from concourse import bacc
    from concourse.masks import make_identity

    f32 = mybir.dt.float32
    f16 = mybir.dt.float16
    AF = mybir.ActivationFunctionType
    OP = mybir.AluOpType

    nc = bacc.Bacc(None, target_bir_lowering=False, debug=False)

    # ---- dram inputs (host pre-formats dtypes/layouts; see kernel()) ------
    latT = nc.dram_tensor("latT", [2, 128, BS], f16, kind="ExternalInput")
    wd_d = nc.dram_tensor("wd", [2, 128, H], f16, kind="ExternalInput")
    w_d = nc.dram_tensor("w", [2, 128, H3], f16, kind="ExternalInput")
    u_d = nc.dram_tensor("u", [4, 128, H3], f16, kind="ExternalInput")
    bx_d = nc.dram_tensor("bx", [H3], f16, kind="ExternalInput")
    bd_d = nc.dram_tensor("bd", [H], f16, kind="ExternalInput")
    b1h_d = nc.dram_tensor("b1h", [128, 4 * HB], f16, kind="ExternalInput")
    out_d = nc.dram_tensor("out", [BS, T, H], f32, kind="ExternalOutput")

    # gate column ranges in the 3H axis (reference order: z, r, h)
    GSL = (slice(0, H), slice(H, 2 * H), slice(2 * H, H3))  # z, r, h

    def as3(tile_or_ap):
        return tile_or_ap[:].rearrange("p (c b) -> p c b", c=4)

    with tile.TileContext(nc) as tc:
        with (
            tc.tile_pool(name="singles", bufs=1) as singles,
            tc.tile_pool(name="work", bufs=3) as work,
            tc.tile_pool(name="hpool", bufs=3) as hpool,
            tc.tile_pool(name="opool", bufs=8) as opool,
            tc.tile_pool(name="psg", bufs=1, space="PSUM") as psg,
        ):
            # ---- load constants ------------------------------------------
            lat = [singles.tile([128, BS], f16, tag=f"lat{j}", name=f"lat{j}")
                   for j in range(2)]
            wd = [singles.tile([128, H], f16, tag=f"wd{j}", name=f"wd{j}")
                  for j in range(2)]
            w = [singles.tile([128, H3], f16, tag=f"w{j}", name=f"w{j}")
                 for j in range(2)]
            for j in range(2):
                nc.sync.dma_start(out=lat[j], in_=latT[j, :, :])
                nc.sync.dma_start(out=wd[j], in_=wd_d[j, :, :])
                nc.sync.dma_start(out=w[j], in_=w_d[j, :, :])
            u = [singles.tile([128, H3], f16, tag=f"u{k}", name=f"u{k}")
                 for k in range(4)]
            for k in range(4):
                nc.sync.dma_start(out=u[k], in_=u_d[k, :, :])

            def bcast(handle, n):
                ap = handle[:]
                return bass.AP(tensor=ap.tensor, offset=ap.offset,
                               ap=[[0, 128], [1, n]])

            xbias = singles.tile([128, H3], f16, tag="xbias")
            nc.gpsimd.dma_start(out=xbias, in_=bcast(bx_d, H3))
            bdt = singles.tile([128, H], f16, tag="bdt")
            nc.gpsimd.dma_start(out=bdt, in_=bcast(bd_d, H))
            # b1h in feature-major half layout: b1h2[p, 64j+b] = b1[2H+128j+p]
            b1h2 = singles.tile([128, 4 * HB], f16, tag="b1h2")
            nc.gpsimd.dma_start(out=b1h2, in_=b1h_d[:, :])

            ident = singles.tile([128, 128], f32, tag="ident")
            make_identity(nc, ident)
            id16 = singles.tile([128, 128], f16, tag="id16")
            nc.scalar.copy(id16, ident)

            # loop/prologue PSUM: one [128, 4*HB] f32 bank per gate per half
            def gate_ps(s):
                return [psg.tile([128, 4 * HB], f32, tag=f"ps{g}_{s}",
                                 name=f"ps{g}_{s}") for g in range(3)]

            # ---- prologue: x-projection and h0, feature-chunked ----------
            # x[g] = latent @ W[:, gate g] + bx[gate g]   (batch-major, f16)
            xg16 = [singles.tile([128, H], f16, tag=f"xg{g}", name=f"xg{g}")
                    for g in range(3)]
            for g in range(3):
                for fc in range(2):
                    ps = psg.tile([128, 4 * HB], f32, tag=f"ps{g}_{fc}",
                                  name=f"pre_x{g}_{fc}")
                    cs = slice(GSL[g].start + 256 * fc,
                               GSL[g].start + 256 * (fc + 1))
                    nc.tensor.matmul(ps, id16, xbias[:, cs], start=True, stop=False)
                    nc.tensor.matmul(ps, lat[0], w[0][:, cs], start=False, stop=False)
                    nc.tensor.matmul(ps, lat[1], w[1][:, cs], start=False, stop=True)
                    nc.scalar.copy(xg16[g][:, 256 * fc : 256 * (fc + 1)], ps)
            # h0 = tanh(latent @ Wd + bd)   (batch-major, f16)
            h016 = singles.tile([128, H], f16, tag="h016")
            for fc in range(2):
                ps = psg.tile([128, 4 * HB], f32, tag=f"ps0_{fc}",
                              name=f"pre_h0_{fc}")
                cs = slice(256 * fc, 256 * (fc + 1))
                nc.tensor.matmul(ps, id16, bdt[:, cs], start=True, stop=False)
                nc.tensor.matmul(ps, lat[0], wd[0][:, cs], start=False, stop=False)
                nc.tensor.matmul(ps, lat[1], wd[1][:, cs], start=False, stop=True)
                nc.scalar.activation(h016[:, cs], ps, AF.Tanh)

            # transpose x and h0 into feature-major half tiles:
            #   xT[g][s][p, 64j+b] = x[g][batch 64s+b, feat 128j+p]
            xT = [[singles.tile([128, 4 * HB], f16, tag=f"xT{g}_{s}",
                                name=f"xT{g}_{s}") for s in range(2)]
                  for g in range(3)]
            for g in range(3):
                for s in range(2):
                    nc.sync.dma_start_transpose(
                        as3(xT[g][s]), xg16[g][HB * s : HB * (s + 1), :])
            hc = []
            for s in range(2):
                t0 = hpool.tile([128, 4 * HB], f16, tag=f"hc{s}", name=f"h0c{s}")
                nc.sync.dma_start_transpose(
                    as3(t0), h016[HB * s : HB * (s + 1), :])
                hc.append(t0)

            # U column slice for gate g, feature-chunk j (stationary operand)
            def u_cols(gsl, j):
                base = gsl.start + 128 * j
                return slice(base, base + 128)

            # ---- steady-state T loop -------------------------------------
            # the out-DMA for step t is emitted at the top of step t+1 so
            # its wait (on the Pool f32 convert) is satisfied when SP's
            # sequencer reaches it -- SP's FIFO never blocks
            pending_out = None
            for t in range(T):
                if pending_out is not None:
                    tp_, buf = pending_out
                    nc.sync.dma_start(out=out_d[:, tp_, :], in_=buf)
                    pending_out = None
                hn = [None, None]
                hcO = opool.tile([128, 4 * BS], f16, tag="hcO")
                for s in range(2):
                    hsrc = hc[s]
                    ps_z, ps_r, ps_h = gate_ps(s)

                    # PE bursts: h first (t1 needs it early), then r, then z
                    for ps, gsl, xin in ((ps_h, GSL[2], b1h2),
                                         (ps_r, GSL[1], xT[1][s]),
                                         (ps_z, GSL[0], xT[0][s])):
                        nc.tensor.matmul(ps, id16, xin, start=True, stop=False)
                        for j in range(4):
                            cs = slice(HB * j, HB * (j + 1))
                            for k in range(4):
                                nc.tensor.matmul(
                                    ps[:, cs], u[k][:, u_cols(gsl, j)],
                                    hsrc[:, HB * k : HB * (k + 1)],
                                    start=False, stop=(k == 3))

                    # ACT: r, z sigmoids then hh tanh (f16 outputs)
                    r16 = work.tile([128, 4 * HB], f16, tag=f"r16_{s}")
                    nc.scalar.activation(r16, ps_r, AF.Sigmoid)
                    z16 = work.tile([128, 4 * HB], f16, tag=f"z16_{s}")
                    nc.scalar.activation(z16, ps_z, AF.Sigmoid)

                    # DVE tail
                    t1 = work.tile([128, 4 * HB], f16, tag=f"t1_{s}")
                    nc.vector.tensor_mul(t1, r16, ps_h)
                    t2 = work.tile([128, 4 * HB], f16, tag=f"t2_{s}")
                    nc.vector.tensor_add(t2, t1, xT[2][s])
                    hh = work.tile([128, 4 * HB], f16, tag=f"hh_{s}")
                    nc.scalar.activation(hh, t2, AF.Tanh)
                    zc = work.tile([128, 4 * HB], f16, tag=f"zc_{s}")
                    nc.vector.tensor_scalar(zc, z16, -1.0, 1.0, OP.mult, OP.add)
                    c1 = work.tile([128, 4 * HB], f16, tag=f"c1_{s}")
                    nc.vector.tensor_mul(c1, z16, hsrc)
                    d = work.tile([128, 4 * HB], f16, tag=f"d_{s}")
                    nc.vector.tensor_mul(d, zc, hh)
                    hnew = hpool.tile([128, 4 * HB], f16, tag=f"hc{s}",
                                      name=f"hn{s}_{t}")
                    nc.vector.tensor_add(hnew, c1, d)
                    # merge into the combined tile for the output transpose
                    nc.vector.tensor_copy(
                        as3(hcO)[:, :, HB * s : HB * (s + 1)], as3(hnew))
                    hn[s] = hnew

                # output path (off the recurrence): f16 transpose to
                # batch-major, GP converts to f32, DMA out
                houtT = opool.tile([128, 4, 128], f16, tag="houtT")
                nc.sync.dma_start_transpose(houtT, hcO)
                hout32 = opool.tile([128, H], f32, tag="hout32")
                nc.gpsimd.tensor_copy(
                    hout32, houtT[:].rearrange("p c b -> p (c b)"))
                pending_out = (t, hout32)

                hc = hn
            tp_, buf = pending_out
            nc.sync.dma_start(out=out_d[:, tp_, :], in_=buf)

    nc.compile()
    return nc


def kernel(latent, Wd, bd, W, U, b, T, _trace=False):
    from concourse.bass_utils import run_bass_kernel_spmd

    latent = np.ascontiguousarray(np.asarray(latent, dtype=np.float32))
    Wd = np.asarray(Wd, dtype=np.float32)
    bd = np.asarray(bd, dtype=np.float32)
    W = np.asarray(W, dtype=np.float32)
    U = np.asarray(U, dtype=np.float32)
    b = np.asarray(b, dtype=np.float32)
    T = int(T)

    key = (T,)
    if key not in _BUILD_CACHE:
        _BUILD_CACHE[key] = _build(T)
    nc = _BUILD_CACHE[key]

    # b[1]'s z/r thirds fold into the x-side bias; the h third stays separate
    bx = b[0].copy()
    bx[: 2 * H] += b[1][: 2 * H]
    b1h = b[1][2 * H :]
    # b1h in feature-major half layout: [p, 64j+b] = b1h[128j+p]
    b1h2 = np.ascontiguousarray(
        np.broadcast_to(b1h.reshape(4, 128).T[:, :, None],
                        (128, 4, HB)).reshape(128, 4 * HB).astype(np.float16))

    u16 = np.ascontiguousarray(U.reshape(4, 128, H3).astype(np.float16))
    wd16 = np.ascontiguousarray(Wd.reshape(2, 128, H).astype(np.float16))
    w16 = np.ascontiguousarray(W.reshape(2, 128, H3).astype(np.float16))
    bx16 = bx.astype(np.float16)
    bd16 = bd.astype(np.float16)

    in_maps = []
    for c in range(NCORES):
        rows = slice(c * BS, (c + 1) * BS)
        latTc = np.ascontiguousarray(
            latent[rows].T.reshape(2, 128, BS).astype(np.float16))
        in_maps.append({
            "latT": latTc, "wd": wd16, "w": w16, "u": u16,
            "bx": bx16, "bd": bd16, "b1h": b1h2,
        })

    res = run_bass_kernel_spmd(nc, in_maps, core_ids=list(range(NCORES)),
                               trace=_trace)
    if _trace and res.exec_time_ns is not None:
        print(f"HW exec time: {res.exec_time_ns} ns")
        if res.instructions_and_trace is not None:
            print(f"trace: {res.instructions_and_trace[1]}")

    out = np.concatenate([r["out"] for r in res.results], axis=0)
    return out
